# revision 12
# baseline (speedup 1.0000x reference)
"""Sparse attention kernel for Trainium2, data-parallel over batch dim N.

Problem: query (N=128, D=128), key/value (T=4096, N, D), lens (N,)
  energy[n,t] = sum_d key[t,n,d] * query[n,d]   (masked t>=lens[n] -> -1e9)
  attention = softmax_t(energy)                  -> (N, T)
  out[n,d] = sum_t attention[n,t] * value[t,n,d] -> (N, D)

Sharding: batch rows split 16-per-core across 8 NeuronCores, no cross-core
communication. Per core, everything is phrased as 128x128 matmuls:
  - host pre-transposes K to [d, t] tiles so energy is lhsT=K^T, rhs=q
  - softmax skips max-subtraction (|energy| <~ 60 << 88, exp stays finite;
    masked exp values are multiplied by a host-built 0/1 mask)
  - Z = sum_t exp via a ones-column matmul accumulated in PSUM
  - context accumulates in PSUM over all 32 t-tiles
  - attention tiles are PE-transposed to row-major before DMA-out
"""

import os
from contextlib import ExitStack

import numpy as np

import concourse.bacc as bacc
import concourse.bass as bass
import concourse.tile as tile
from concourse import mybir
from concourse.bass_utils import run_bass_kernel_spmd

N, T, D = 128, 4096, 128
NCORES = 8
NC = N // NCORES          # 16 batch rows per core
P = 128                   # timesteps per tile / partition count
J = T // P                # 32 t-tiles
F32 = mybir.dt.float32

_CACHE = {}
last_results = None       # BassKernelResults of most recent run (for test.py)


def _build():
    nc = bacc.Bacc(None)
    ktd = nc.dram_tensor("kt", (J, P, NC * P), F32, kind="ExternalInput")
    vd = nc.dram_tensor("v", (J, P, NC * D), F32, kind="ExternalInput")
    qtd = nc.dram_tensor("qt", (P, NC), F32, kind="ExternalInput")
    md = nc.dram_tensor("mask", (P, J * NC), F32, kind="ExternalInput")
    cd = nc.dram_tensor("cst", (P, P + 1), F32, kind="ExternalInput")
    od = nc.dram_tensor("out", (NC, D), F32, kind="ExternalOutput")
    ad = nc.dram_tensor("att", (NC, T), F32, kind="ExternalOutput")

    with tile.TileContext(nc) as tc, ExitStack() as ctx:
        singles = ctx.enter_context(tc.tile_pool(name="singles", bufs=1))
        esb = ctx.enter_context(tc.tile_pool(name="esb", bufs=2))
        kv = ctx.enter_context(tc.tile_pool(name="kv", bufs=3))
        pp = ctx.enter_context(tc.tile_pool(name="pp", bufs=J))
        outs = ctx.enter_context(tc.tile_pool(name="outs", bufs=4))
        pe_ps = ctx.enter_context(tc.tile_pool(name="pe", bufs=2, space="PSUM"))
        cj_ps = ctx.enter_context(tc.tile_pool(name="cj", bufs=2, space="PSUM"))
        z_ps = ctx.enter_context(tc.tile_pool(name="z", bufs=1, space="PSUM"))
        tr_ps = ctx.enter_context(tc.tile_pool(name="tr", bufs=2, space="PSUM"))

        cst_sb = singles.tile([P, P + 1], F32)
        nc.sync.dma_start(out=cst_sb, in_=cd[:, :])
        ident = cst_sb[:, 0:P]
        ones = cst_sb[:, P:P + 1]
        qt_sb = singles.tile([P, NC], F32)
        nc.sync.dma_start(out=qt_sb, in_=qtd[:, :])
        mask_sb = singles.tile([P, J * NC], F32)
        nc.sync.dma_start(out=mask_sb, in_=md[:, :])

        # ctx accumulates in SBUF: 16 interleaved multi-j accumulation chains
        # inside one PSUM tile are illegal (start=True zeroes the whole bank
        # region, breaking sibling columns' pending groups). Per-j matmuls
        # use instant groups (start&stop) and DVE adds them into SBUF.
        ctx_acc = singles.tile([P, NC], F32)
        z_psum = z_ps.tile([NC, 1], F32)
        p_tiles = []
        for j in range(J):
            kt_t = kv.tile([P, NC * P], F32, tag="kt")
            nc.sync.dma_start(out=kt_t, in_=ktd[j, :, :])
            v_t = kv.tile([P, NC * D], F32, tag="v")
            nc.sync.dma_start(out=v_t, in_=vd[j, :, :])

            e_ps = pe_ps.tile([P, NC], F32)
            for nn in range(NC):
                nc.tensor.matmul(
                    e_ps[:, nn:nn + 1],
                    kt_t[:, nn * P:(nn + 1) * P],
                    qt_sb[:, nn:nn + 1],
                    start=True, stop=True,
                )
            # e_ps -> SBUF on DVE (so the PSUM slot release rides the same
            # PE<-DVE sem chain as p_j and the matmul stays at <=2 waits),
            # exp on ACT, mask-mul back on DVE as p_j's final writer.
            e_sb = esb.tile([P, NC], F32, tag="esb")
            nc.vector.tensor_copy(e_sb, e_ps)
            p_j = pp.tile([P, NC], F32, tag="p")
            nc.scalar.activation(p_j, e_sb, mybir.ActivationFunctionType.Exp)
            nc.vector.tensor_mul(p_j, p_j, mask_sb[:, j * NC:(j + 1) * NC])

            nc.tensor.matmul(z_psum, p_j, ones, start=(j == 0), stop=(j == J - 1))
            cj = cj_ps.tile([P, NC], F32, tag="cj")
            for nn in range(NC):
                nc.tensor.matmul(
                    cj[:, nn:nn + 1],
                    v_t[:, nn * D:(nn + 1) * D],
                    p_j[:, nn:nn + 1],
                    start=True, stop=True,
                )
            if j == 0:
                nc.vector.tensor_copy(ctx_acc, cj)
            else:
                nc.vector.tensor_add(ctx_acc, ctx_acc, cj)
            p_tiles.append(p_j)

        rz = singles.tile([NC, 1], F32)
        nc.vector.reciprocal(rz, z_psum)

        outT = tr_ps.tile([NC, P], F32, tag="tr")
        nc.tensor.transpose(outT, ctx_acc, ident)
        out_sb = outs.tile([NC, D], F32, tag="o")
        nc.vector.tensor_scalar_mul(out_sb, outT, rz)
        nc.sync.dma_start(out=od[:, :], in_=out_sb)

        for j in range(J):
            trp = tr_ps.tile([NC, P], F32, tag="tr")
            nc.tensor.transpose(trp, p_tiles[j], ident)
            a_sb = outs.tile([NC, P], F32, tag="a")
            nc.vector.tensor_scalar_mul(a_sb, trp, rz)
            nc.sync.dma_start(out=ad[:, j * P:(j + 1) * P], in_=a_sb)

    # Bacc.finalize runs compile(): register allocation + splitting sync
    # waits to <=1 per instruction (the Matmult LW ISA limit). The axon
    # PJRT path binds the primitive directly and never finalizes for us.
    nc.finalize()
    return nc


_CST = None


def _prep_core(query, key, value, lens, c):
    n0 = c * NC
    qt = np.ascontiguousarray(query[n0:n0 + NC].T)                    # (128, 16)
    k = key[:, n0:n0 + NC, :].reshape(J, P, NC, D)
    kt = np.ascontiguousarray(k.transpose(0, 3, 2, 1)).reshape(J, D, NC * P)
    v = np.ascontiguousarray(value[:, n0:n0 + NC, :]).reshape(J, P, NC * D)
    lens_c = lens[n0:n0 + NC].astype(np.int64)
    t_idx = np.arange(T, dtype=np.int64).reshape(J, P)
    m = (t_idx[:, :, None] < lens_c[None, None, :]).astype(np.float32)
    mask = np.ascontiguousarray(m.transpose(1, 0, 2)).reshape(P, J * NC)
    return {"kt": kt, "v": v, "qt": qt, "mask": mask, "cst": _CST}


def kernel(query, key, value, lens_for_attention):
    global _CST, last_results
    query = np.asarray(query, dtype=np.float32)
    key = np.asarray(key, dtype=np.float32)
    value = np.asarray(value, dtype=np.float32)
    lens = np.asarray(lens_for_attention)

    if _CST is None:
        _CST = np.concatenate(
            [np.eye(P, dtype=np.float32), np.ones((P, 1), np.float32)], axis=1
        )
    if "nc" not in _CACHE:
        _CACHE["nc"] = _build()
    nc = _CACHE["nc"]

    in_maps = [_prep_core(query, key, value, lens, c) for c in range(NCORES)]
    trace = bool(os.environ.get("KTRACE"))
    last_results = run_bass_kernel_spmd(
        nc, in_maps, core_ids=list(range(NCORES)), trace=trace
    )
    res = last_results.results

    out = np.empty((N, D), dtype=np.float32)
    att = np.empty((N, T), dtype=np.float32)
    for c in range(NCORES):
        out[c * NC:(c + 1) * NC] = res[c]["out"]
        att[c * NC:(c + 1) * NC] = res[c]["att"]
    return out, att


# revision 40
# speedup vs baseline: 2.4797x; 2.4797x over previous
"""Sparse attention kernel for Trainium2, data-parallel over batch dim N.

Problem: query (N=128, D=128), key/value (T=4096, N, D), lens (N,)
  energy[n,t] = sum_d key[t,n,d] * query[n,d]   (masked t>=lens[n] -> -1e9)
  attention = softmax_t(energy)                  -> (N, T)
  out[n,d] = sum_t attention[n,t] * value[t,n,d] -> (N, D)

Sharding: batch rows split 16-per-core across 8 NeuronCores, no cross-core
communication. Per core:
  - host pre-transposes K to [d, t] tiles so energy is lhsT=K^T, rhs=q
  - the length mask is folded into K host-side (masked rows rewritten so
    energy == -1e6 -> exp underflows to exact 0, like the reference's -1e9)
  - softmax skips max-subtraction (|energy| <~ 60 << 88, exp stays finite)
  - Z = sum_t exp via a ones-column matmul PSUM-accumulated over t-tiles
  - ctx: 4 p-columns stationary x 4 V-blocks streaming -> [4,512]
    cross-product tiles PSUM-accumulated over t-tiles; diagonal extracted
    once at the end with partition-strided DMAs (compute engines cannot
    address partition bases other than 0/32/64)
  - attention tiles are PE-transposed to row-major before DMA-out
"""

import os
from contextlib import ExitStack

import numpy as np

import concourse.bacc as bacc
import concourse.bass as bass
import concourse.tile as tile
from concourse import mybir
from concourse.bass_utils import run_bass_kernel_spmd

N, T, D = 128, 4096, 128
NCORES = 8
NC = N // NCORES          # 16 batch rows per core
P = 128                   # timesteps per tile / partition count
J = T // P                # 32 t-tiles
BLK = 1                   # t-tiles per DMA (fewer, larger blocking loads)
KV_BUFS = 6
F32 = mybir.dt.float32

_CACHE = {}
last_results = None       # BassKernelResults of most recent run (for test.py)


def _build(reps=1):
    nc = bacc.Bacc(None)
    kvd = nc.dram_tensor("kv", (J, P, NC * P + NC * D), F32, kind="ExternalInput")
    qtd = nc.dram_tensor("qt", (P, NC), F32, kind="ExternalInput")
    cd = nc.dram_tensor("cst", (P, P + 1), F32, kind="ExternalInput")
    od = nc.dram_tensor("out", (NC, D), F32, kind="ExternalOutput")
    ad = nc.dram_tensor("att", (NC, T), F32, kind="ExternalOutput")

    with tile.TileContext(nc) as tc, ExitStack() as ctx:
        singles = ctx.enter_context(tc.tile_pool(name="singles", bufs=1))
        esb = ctx.enter_context(tc.tile_pool(name="esb", bufs=2))
        kv = ctx.enter_context(tc.tile_pool(name="kv", bufs=KV_BUFS))
        pp = ctx.enter_context(tc.tile_pool(name="pp", bufs=J))
        outs = ctx.enter_context(tc.tile_pool(name="outs", bufs=4))
        pe_ps = ctx.enter_context(tc.tile_pool(name="pe", bufs=2, space="PSUM"))
        cj_ps = ctx.enter_context(tc.tile_pool(name="cj", bufs=1, space="PSUM"))
        z_ps = ctx.enter_context(tc.tile_pool(name="z", bufs=1, space="PSUM"))
        tr_ps = ctx.enter_context(tc.tile_pool(name="tr", bufs=1, space="PSUM"))

        cst_sb = singles.tile([P, P + 1], F32)
        nc.sync.dma_start(out=cst_sb, in_=cd[:, :])
        ident = cst_sb[:, 0:P]
        ones = cst_sb[:, P:P + 1]
        qt_sb = singles.tile([P, NC], F32)
        nc.sync.dma_start(out=qt_sb, in_=qtd[:, :])
        for _rep in range(reps):
            _kernel_body(nc, tc, singles, esb, kv, pp, outs, pe_ps, cj_ps, z_ps,
                         tr_ps, cst_sb, ident, ones, qt_sb,
                         kvd, od, ad)

    # Bacc.finalize runs compile(): register allocation + splitting sync
    # waits to <=1 per instruction (the Matmult LW ISA limit). The axon
    # PJRT path binds the primitive directly and never finalizes for us.
    nc.finalize()
    return nc


def _kernel_body(nc, tc, singles, esb, kv, pp, outs, pe_ps, cj_ps, z_ps, tr_ps,
                 cst_sb, ident, ones, qt_sb, kvd, od, ad):
    if True:
        z_psum = z_ps.tile([NC, 1], F32)
        cjs = [
            cj_ps.tile([4, 512], F32, name=f"cj{g}", tag=f"cj{g}") for g in range(4)
        ]
        p_tiles = []
        for jb in range(J):
            kv_t = kv.tile([P, NC * P + NC * D], F32, tag="kv")
            nc.sync.dma_start(out=kv_t, in_=kvd[jb, :, :])

            for jj in range(1):
                j = jb
                kt_j = kv_t[:, :NC * P]
                v_j = kv_t[:, NC * P:]
                e_ps = pe_ps.tile([P, NC], F32)
                for nn in range(NC):
                    nc.tensor.matmul(
                        e_ps[:, nn:nn + 1],
                        kt_j[:, nn * P:(nn + 1) * P],
                        qt_sb[:, nn:nn + 1],
                        start=True, stop=True,
                    )
                # exp straight from PSUM on ACT; masking folded into K
                # host-side (Bacc splits >1-wait matmuls via EventSemaphore).
                p_j = pp.tile([P, NC], F32, tag="p")
                nc.scalar.activation(p_j, e_ps, mybir.ActivationFunctionType.Exp)

                nc.tensor.matmul(z_psum, p_j, ones,
                                 start=(j == 0), stop=(j == J - 1))
                # ctx flipped: 4 p columns stationary (2KB LW) x 4 V blocks
                # streaming (512 cols) -> [4,512] cross-product tile at
                # partition 0 (legal base), PSUM-accumulated over j as a
                # single-region chain. Only the diagonal [r, r*128:+128] is
                # real ctx; extracted once after the loop. Balances the two
                # PE SBUF read ports (K on LW, V on rhs).
                for g in range(4):
                    nc.tensor.matmul(
                        cjs[g],
                        p_j[:, 4 * g:4 * g + 4],
                        v_j[:, 4 * g * D:(4 * g + 4) * D],
                        start=(j == 0), stop=(j == J - 1),
                    )
                p_tiles.append(p_j)

        rz = singles.tile([NC, 1], F32)
        nc.vector.reciprocal(rz, z_psum)

        # Compute engines can't address partition bases other than 0/32/64,
        # so the cross-product diagonal is gathered with partition-strided
        # DMAs instead: PSUM -> ctx_sb (DVE, base 0), then per-r DMA of the
        # diagonal blocks into diag (nn = 4g + r), then one legal scale.
        ctx_sb = singles.tile([4, 4 * 512], F32)
        for g in range(4):
            nc.vector.tensor_copy(ctx_sb[:, g * 512:(g + 1) * 512], cjs[g])
        diag = singles.tile([NC, D], F32)
        nc.vector.memset(diag, 0.0)
        for r in range(4):
            src = ctx_sb[r:r + 1, :].rearrange(
                "p (g x) -> p g x", g=4)[:, :, r * D:(r + 1) * D]
            dst = diag.rearrange("(a b) d -> b a d", b=4)[r, :, :]
            nc.sync.dma_start(out=dst, in_=src)
        out_sb = outs.tile([NC, D], F32, tag="o")
        nc.vector.tensor_scalar_mul(out_sb, diag, rz)
        nc.sync.dma_start(out=od[:, :], in_=out_sb)

        for j in range(J):
            trp = tr_ps.tile([NC, P], F32, tag="tr")
            nc.tensor.transpose(trp, p_tiles[j], ident)
            a_sb = outs.tile([NC, P], F32, tag="a")
            nc.vector.tensor_scalar_mul(a_sb, trp, rz)
            nc.sync.dma_start(out=ad[:, j * P:(j + 1) * P], in_=a_sb)


_CST = None


def _prep_core(query, key, value, lens, c):
    n0 = c * NC
    qt = np.ascontiguousarray(query[n0:n0 + NC].T)                    # (128, 16)
    # Fold the length mask into K: masked rows are rewritten so that
    # energy = k.q == -1e6 exactly, and exp(-1e6) underflows to 0 in f32
    # (reference gets 0 the same way via its -1e9 substitution). Removes
    # the mask tensor and the per-tile mask multiply from the device.
    k = np.ascontiguousarray(key[:, n0:n0 + NC, :])                   # (T, 16, D)
    for nn in range(NC):
        ln = int(lens[n0 + nn])
        if ln < T:
            qn = query[n0 + nn]
            k[ln:, nn, :] = qn * (-1e6 / float(qn @ qn))
    kt = np.ascontiguousarray(
        k.reshape(J, P, NC, D).transpose(0, 3, 2, 1)).reshape(J, D, NC * P)
    v = np.ascontiguousarray(value[:, n0:n0 + NC, :]).reshape(J, P, NC * D)
    kvp = np.concatenate([kt, v], axis=2)
    return {"kv": kvp, "qt": qt, "cst": _CST}


def kernel(query, key, value, lens_for_attention):
    global _CST, last_results
    query = np.asarray(query, dtype=np.float32)
    key = np.asarray(key, dtype=np.float32)
    value = np.asarray(value, dtype=np.float32)
    lens = np.asarray(lens_for_attention)

    if _CST is None:
        _CST = np.concatenate(
            [np.eye(P, dtype=np.float32), np.ones((P, 1), np.float32)], axis=1
        )
    if "nc" not in _CACHE:
        _CACHE["nc"] = _build()
    nc = _CACHE["nc"]

    in_maps = [_prep_core(query, key, value, lens, c) for c in range(NCORES)]
    trace = bool(os.environ.get("KTRACE"))
    last_results = run_bass_kernel_spmd(
        nc, in_maps, core_ids=list(range(NCORES)), trace=trace
    )
    res = last_results.results

    out = np.empty((N, D), dtype=np.float32)
    att = np.empty((N, T), dtype=np.float32)
    for c in range(NCORES):
        out[c * NC:(c + 1) * NC] = res[c]["out"]
        att[c * NC:(c + 1) * NC] = res[c]["att"]
    return out, att
